# revision 10
# baseline (speedup 1.0000x reference)
"""ANFIS forward kernel for Trainium2 (8 NeuronCores, data-parallel over batch).

Problem shapes (hardcoded): B=16384, R=512 rules, F=32 features, O=8 outputs.

Math (identical to the reference, re-associated for the PE):
  a[r]            = -1 / (2*(|w[r]|+0.1)^2)
  v[r,b]          = a[r] * (x2[b] + c2[r] - 2*cross[b,r])      (= -dist/(2 s^2))
  f[r,b]          = exp(v[r,b])                                 (UNnormalized)
  H[b, o*32+f]    = sum_r f[r,b] * W[r,f,o]
  H[b, 256+o]     = sum_r f[r,b] * cb[r,o]
  H[b, 264]       = sum_r f[r,b]                  (= S[b])
  out[b,o]        = (sum_f x[b,f]*H[b,o*32+f] + H[b,256+o]) / (S[b]+1e-8)

Device mapping per core (2048 batch rows):
  MM1 (PE):  psum[128r, 512b] = daug[:, rc]^T @ xat_tile   (K=34: x rows, ones, x2)
             -> firing computed directly TRANSPOSED (rules on partitions),
                which is exactly the layout MM2 needs as its stationary operand.
  EXP (ACT): firing tile = exp(psum)
  MM2 (PE):  psum_h[128b, 265] += firing[rc][:, j128]^T @ wr2[rc]   (4 chained)
  Combine (DVE): broadcast mul (x repeated over o via 0-stride AP) + reduce_sum
             + bias add, then out = numerator * reciprocal(S + 1e-8).

Notes:
  * Built on bacc.Bacc + nc.compile(): generate_event_semaphores legalizes the
    1-wait-per-instruction TRN2 limit, so multi-dep matmuls/DMAs are fine.
  * tensor_tensor_reduce hangs this hardware/runtime combination - do not use.
  * MM2 runs in float32r (TF32-like, 1 cycle/row at N>=256 vs 4 for fp32);
    firing is in [0,1] and W is small, measured accuracy cost is acceptable.
    MM1 stays fp32: its output feeds exp(), where input error is amplified.
"""

import numpy as np

import concourse.bacc as bacc
import concourse.bass as bass
import concourse.tile as tile
from concourse import mybir
from concourse.bass_utils import run_bass_kernel_spmd

B, R, F, O = 16384, 512, 32, 8
NCORES = 8
BL = B // NCORES           # 2048 batch rows per core
GW = 512                   # batch-group width for MM1 streaming
NG = BL // GW              # 4 groups per core
NT = BL // 128             # 16 b128 tiles per core
K1 = F + 2                 # 34 = x rows + ones row + x2 row
NC2 = O * F + O + 2        # 266 = (o,f) block + bias + ones + pad (fp32r needs even dst)
RC = R // 128              # 4 rule chunks

F32 = mybir.dt.float32
F32R = mybir.dt.float32r
EXP = mybir.ActivationFunctionType.Exp

MM1_DT = F32               # exact: feeds exp()
MM2_DT = F32R              # fast path: 1 cycle/row at N>=256

_PROGRAM = None


def _build_program():
    nc = bacc.Bacc()
    xat_d = nc.declare_dram_parameter("xat", [K1, BL], F32, isOutput=False)
    x_d = nc.declare_dram_parameter("x", [BL, F], F32, isOutput=False)
    daug_d = nc.declare_dram_parameter("daug", [K1, R], F32, isOutput=False)
    wr2_d = nc.declare_dram_parameter("wr2", [RC, 128, NC2], MM2_DT, isOutput=False)
    out_d = nc.declare_dram_parameter("out", [BL, O], F32, isOutput=True)

    with tile.TileContext(nc) as tc:
        with (
            tc.tile_pool(name="one", bufs=1) as one,
            tc.tile_pool(name="ft", bufs=2 * RC) as ft_pool,
            tc.tile_pool(name="scr", bufs=2) as scr_pool,
            tc.tile_pool(name="sm", bufs=4) as sm_pool,
            tc.tile_pool(name="ps1", bufs=2, space="PSUM") as ps1_pool,
            tc.tile_pool(name="psh", bufs=4, space="PSUM") as psh_pool,
        ):
            # single-shot loads (bufs=1: written once, never recycled)
            d_sb = one.tile([K1, R], F32, tag="daug")
            nc.sync.dma_start(out=d_sb[:], in_=daug_d[:])
            w_sb = one.tile([128, RC, NC2], MM2_DT, tag="wr2")
            nc.sync.dma_start(out=w_sb[:], in_=wr2_d.rearrange("rc p c -> p rc c"))
            xa_sb = one.tile([K1, BL], F32, tag="xa")
            nc.sync.dma_start(out=xa_sb[:], in_=xat_d[:])
            x_all = one.tile([128, NT, F], F32, tag="xall")
            nc.sync.dma_start(out=x_all[:], in_=x_d.rearrange("(t p) f -> p t f", p=128))

            out_all = one.tile([128, NT, O], F32, tag="outall")

            for g in range(NG):
                fts = []
                for rc in range(RC):
                    ps1 = ps1_pool.tile([128, GW], F32, tag="ps1")
                    nc.tensor.matmul(
                        ps1[:],
                        d_sb[:, rc * 128:(rc + 1) * 128],
                        xa_sb[:, g * GW:(g + 1) * GW],
                        start=True,
                        stop=True,
                    )
                    ft = ft_pool.tile([128, GW], MM2_DT, tag="ft")
                    nc.scalar.activation(ft[:], ps1[:], EXP)
                    fts.append(ft)

                for j in range(GW // 128):
                    t = g * (GW // 128) + j
                    psh = psh_pool.tile([128, NC2], F32, tag="psh")
                    for rc in range(RC):
                        nc.tensor.matmul(
                            psh[:],
                            fts[rc][:, j * 128:(j + 1) * 128],
                            w_sb[:, rc, :],
                            start=(rc == 0),
                            stop=(rc == RC - 1),
                        )

                    # numerator[p, o] = sum_f x[p,f]*H[p,(o,f)] + bias[p,o];
                    # x is broadcast across o with a 0-stride middle AP dim.
                    xa = x_all[:, t, :]
                    xb = bass.AP(
                        tensor=xa.tensor,
                        offset=xa.offset,
                        ap=[xa.ap[0], [0, O], xa.ap[1]],
                    )
                    scratch = scr_pool.tile([128, O, F], F32, tag="scr")
                    nc.vector.tensor_mul(
                        scratch[:],
                        psh[:, 0:O * F].rearrange("p (o f) -> p o f", o=O),
                        xb,
                    )
                    osum = sm_pool.tile([128, O], F32, tag="osum")
                    nc.vector.reduce_sum(
                        out=osum[:], in_=scratch[:], axis=mybir.AxisListType.X
                    )
                    num_t = sm_pool.tile([128, O], F32, tag="num")
                    nc.vector.tensor_add(num_t[:], osum[:], psh[:, O * F:O * F + O])

                    sden = sm_pool.tile([128, 1], F32, tag="sden")
                    nc.vector.tensor_scalar_add(sden[:], psh[:, O * F + O:O * F + O + 1], 1e-8)
                    rec = sm_pool.tile([128, 1], F32, tag="rec")
                    nc.vector.reciprocal(rec[:], sden[:])
                    nc.vector.tensor_scalar_mul(out_all[:, t, :], num_t[:], rec[:])

            nc.sync.dma_start(
                out=out_d.rearrange("(t p) o -> p t o", p=128), in_=out_all[:]
            )
    nc.compile()
    return nc


def get_program():
    global _PROGRAM
    if _PROGRAM is None:
        _PROGRAM = _build_program()
    return _PROGRAM


def make_in_maps(inputs, centers, widths, consequent_w, consequent_b):
    x = np.ascontiguousarray(np.asarray(inputs, dtype=np.float32))
    c64 = np.asarray(centers, dtype=np.float64)
    w64 = np.asarray(widths, dtype=np.float64)
    W = np.asarray(consequent_w, dtype=np.float32)
    cb = np.asarray(consequent_b, dtype=np.float32)

    s = np.abs(w64) + 0.1
    a = -1.0 / (2.0 * s * s)                       # [R]

    daug = np.empty((K1, R), dtype=np.float64)
    daug[:F] = (c64 * (-2.0 * a)[:, None]).T       # -2*c[r,f]*a[r]
    daug[F] = (c64 * c64).sum(axis=1) * a          # c2[r]*a[r]  (pairs with ones)
    daug[F + 1] = a                                # a[r]        (pairs with x2)
    daug = daug.astype(np.float32)

    wr2 = np.zeros((R, NC2), dtype=np.float32)
    wr2[:, :F * O] = W.transpose(0, 2, 1).reshape(R, F * O)   # col = o*32+f
    wr2[:, F * O:F * O + O] = cb
    wr2[:, F * O + O] = 1.0                                   # S column; last col stays 0 (pad)
    wr2 = np.ascontiguousarray(wr2.reshape(RC, 128, NC2))

    x2 = np.einsum("bf,bf->b", x, x).astype(np.float32)
    xat = np.empty((K1, B), dtype=np.float32)
    xat[:F] = x.T
    xat[F] = 1.0
    xat[F + 1] = x2

    in_maps = []
    for ci in range(NCORES):
        sl = slice(ci * BL, (ci + 1) * BL)
        in_maps.append({
            "xat": np.ascontiguousarray(xat[:, sl]),
            "x": np.ascontiguousarray(x[sl]),
            "daug": daug,
            "wr2": wr2,
        })
    return in_maps


def _axon_reset():
    """Recover a wedged NeuronCore (NRT_EXEC_UNIT_UNRECOVERABLE) via the axon
    client's reset entry point.  Best-effort: silently skipped off-axon."""
    try:
        import ctypes
        import time

        import jax

        jax.devices()
        lib = ctypes.CDLL("/opt/axon/libaxon_pjrt.so")
        lib.axon_reset.restype = ctypes.c_int64
        lib.axon_reset()
        time.sleep(2)
    except Exception:
        pass


def kernel(inputs, centers, widths, consequent_w, consequent_b):
    nc = get_program()
    in_maps = make_in_maps(inputs, centers, widths, consequent_w, consequent_b)
    last_err = None
    for attempt in range(3):
        try:
            res = run_bass_kernel_spmd(nc, in_maps, list(range(NCORES))).results
            return np.concatenate([r["out"] for r in res], axis=0)
        except Exception as e:  # wedged device -> reset + retry
            last_err = e
            _axon_reset()
    raise last_err
